# revision 29
# baseline (speedup 1.0000x reference)
"""Multi-head causal attention (B=4, T=2048, C=1024, H=16, D=64) on 8 trn2 cores.

Sharding: tensor-parallel over heads within batch core-pairs.
  core c -> batch b = c//2, heads hoff..hoff+7 where hoff = (c%2)*8.

Per-core pipeline (all matmul operands bf16, accumulation f32 in PSUM):
  - projections per t-slab (Q^T/K^T head-pair packed to 128 partitions; V
    head-packed with a ones column folded in for free softmax sums);
    slab tt+1's projections are interleaved into slab tt's attention so the
    PE fills the exp-latency gaps (attention is ACT-bound).
  - causal attention per head in S^T = [j, i] orientation, exp without
    max-subtraction (scores ~N(0, 0.25^2), safe); merged affine_select
    causal masks (one per diagonal chunk-pair).
  - AV in flipped orientation: stationary = P^T chunk [128 k, 128 q],
    moving = V [128 k, 65] -> O accumulates as [q, d|l] in PSUM; fully
    masked (q-chunk < key-chunk) matmuls skipped. Softmax normalization is
    then a per-partition tensor_scalar multiply; O^T rebuilt with PE
    transposes for the output projection.
  - output projection to partial y^T [1024 c', 2048 t] (+ bo/2) in bf16
  - pairwise ReduceScatter (bf16, per t-slab) sums partner partials; core
    even keeps c' 0:512, odd keeps c' 512:1024.
Host reassembles the [B, T, C] f32 output by transposing/concatenating.
"""

import numpy as np

import concourse.bass as bass
import concourse.mybir as mybir
from concourse import bacc, masks
from concourse.tile import TileContext
from concourse.bass_utils import run_bass_kernel_spmd

F32 = mybir.dt.float32
BF16 = mybir.dt.bfloat16

B, T, C = 4, 2048, 1024
H, D = 16, 64
HC = 8           # heads per core
NPAIR = HC // 2  # head pairs (QK packing)
CCn = C // 128   # 8 contraction chunks
TTn = T // 512   # 4 query slabs of 512
JCn = T // 128   # 16 key chunks of 128
N_CORES = 8
RG = [[0, 1], [2, 3], [4, 5], [6, 7]]


def build_nc(with_rs: bool = True):
    nc = bacc.Bacc(None, target_bir_lowering=False)

    xT = nc.declare_dram_parameter("xT", [C, T], BF16, isOutput=False)
    wq = nc.declare_dram_parameter("wq", [C, 512], BF16, isOutput=False)
    wk = nc.declare_dram_parameter("wk", [C, 512], BF16, isOutput=False)
    wv = nc.declare_dram_parameter("wv", [C, 512], BF16, isOutput=False)
    wot = nc.declare_dram_parameter("wot", [512, C], BF16, isOutput=False)
    bo2 = nc.declare_dram_parameter("bo2", [128, 8], F32, isOutput=False)
    y = nc.declare_dram_parameter("y", [TTn, 512, 512], BF16, isOutput=True)

    with TileContext(nc) as tc:
        with (
            tc.tile_pool(name="persist", bufs=1) as pp,
            tc.tile_pool(name="psum", bufs=1, space="PSUM") as psum,
            tc.tile_pool(name="dram", bufs=1, space="DRAM") as dram,
        ):
            # ---- persistent tiles ----
            qt = [pp.tile([128, T], BF16, tag=f"qt{p}", name=f"qt{p}")
                  for p in range(NPAIR)]
            kt = [pp.tile([128, T], BF16, tag=f"kt{p}", name=f"kt{p}")
                  for p in range(NPAIR)]
            # V chunks: 8 heads * 65 cols (64 d + ones col for softmax sums)
            v = [pp.tile([128, 65 * HC], BF16, tag=f"v{j}", name=f"v{j}")
                 for j in range(JCn)]
            ot = [pp.tile([128, T], BF16, tag=f"ot{p}", name=f"ot{p}")
                  for p in range(NPAIR)]
            # weights as single big tiles, loaded with one DMA each
            # (HWDGE fixed overhead dominates many small DMAs at startup);
            # chunk cc lives at cols [cc*512:(cc+1)*512]
            wqt = pp.tile([128, CCn * 512], BF16, tag="wqB", name="wqt")
            wkt = pp.tile([128, CCn * 512], BF16, tag="wkB", name="wkt")
            wvt = pp.tile([128, CCn * 512], BF16, tag="wvB", name="wvt")
            wot_t = pp.tile([128, 4 * C], BF16, tag="wotB", name="wot_t")
            ones8 = pp.tile([128, HC], BF16, tag="ones8")
            nc.vector.memset(ones8[:], 1.0)
            ident = pp.tile([128, 128], BF16, tag="ident")
            masks.make_identity(nc, ident[:])
            bo_sb = pp.tile([128, 8], F32, tag="bo_sb")
            # tt3 outproj split: pairs 0-1 (+bias) accumulate here during
            # h5/h6 so the tail only runs pairs 2-3
            acc = pp.tile([128, 8 * 512], BF16, tag="acc", name="acc")

            y_part = dram.tile([TTn, 1024, 512], BF16)
            rs_out = dram.tile([TTn, 512, 512], BF16)

            # ---- startup DMAs: one batched transfer per tensor ----
            xts_store = {}

            def big_dma(dst, src, nrows):
                nc.sync.dma_start(
                    out=dst[:].rearrange("p (cc n) -> p cc n", n=nrows),
                    in_=src.rearrange("(cc p) n -> p cc n", p=128),
                )

            def issue_xts(tt):
                i0 = tt * 512
                xts = pp.tile([128, CCn * 512], BF16, tag="xtB", bufs=2,
                              name=f"xts_{tt}")
                big_dma(xts, xT[:, i0:i0 + 512], 512)
                xts_store[tt] = xts

            # wq/x0 interleaved in halves so the first Q matmuls start
            # ~3us earlier than with whole-tensor transfers
            xts0 = pp.tile([128, CCn * 512], BF16, tag="xtB", bufs=2,
                           name="xts_0")
            xts_store[0] = xts0
            for hf in range(4):
                c0 = hf * 2
                nc.sync.dma_start(
                    out=wqt[:, c0 * 512:(c0 + 2) * 512].rearrange(
                        "p (cc n) -> p cc n", n=512),
                    in_=wq[c0 * 128:(c0 + 2) * 128, :].rearrange(
                        "(cc p) n -> p cc n", p=128),
                )
                nc.sync.dma_start(
                    out=xts0[:, c0 * 512:(c0 + 2) * 512].rearrange(
                        "p (cc n) -> p cc n", n=512),
                    in_=xT[c0 * 128:(c0 + 2) * 128, 0:512].rearrange(
                        "(cc p) n -> p cc n", p=128),
                )
            big_dma(wkt, wk[:], 512)
            big_dma(wvt, wv[:], 512)
            big_dma(wot_t, wot[:], C)
            nc.sync.dma_start(out=bo_sb[:], in_=bo2[:])

            # ---- projection emission (phase A), half-group units ----
            def emit_a_half(tt, gi, half):
                """gi 0-1: Q pair-halves; 2-3: K; 4-7: V chunks.
                half 0/1 emits ~1.7/0.85us of PE work (yps tag, not st, so
                this filler can't stall the QK exp-psum rotation)."""
                i0 = tt * 512
                xts = xts_store[tt]
                if gi < 4:
                    wt, dst = (wqt, qt) if gi < 2 else (wkt, kt)
                    p = 2 * (gi % 2) + half
                    ps = psum.tile([128, 512], F32, tag="yps", bufs=2,
                                   name=f"aps{tt}_{gi}_{half}")
                    for cc in range(CCn):
                        nc.tensor.matmul(
                            ps[:],
                            wt[:, cc * 512 + p * 128:cc * 512 + (p + 1) * 128],
                            xts[:, cc * 512:(cc + 1) * 512],
                            start=(cc == 0), stop=(cc == CCn - 1),
                            skip_group_check=True,
                        )
                    nc.vector.tensor_copy(dst[p][:, i0:i0 + 512], ps[:])
                else:
                    jc = 4 * tt + (gi - 4)
                    jl = jc * 128 - i0
                    if half == 0:
                        ps = psum.tile([128, 512], F32, tag="yps", bufs=2,
                                       name=f"vps{jc}")
                        vps_store[jc] = ps
                    else:
                        ps = vps_store.pop(jc)
                    g = half
                    for cc in range(CCn):
                        nc.tensor.matmul(
                            ps[:, g * 256:(g + 1) * 256],
                            xts[:, cc * 512 + jl:cc * 512 + jl + 128],
                            wvt[:, cc * 512 + g * 256:cc * 512 + (g + 1) * 256],
                            start=(cc == 0), stop=(cc == CCn - 1),
                            skip_group_check=True,
                        )
                    if half == 1:
                        vre = v[jc][:].rearrange("p (h e) -> p h e", h=HC, e=65)
                        nc.vector.tensor_copy(vre[:, :, 0:64], ps[:])
                        nc.vector.tensor_copy(vre[:, :, 64:65], ones8[:])

            vps_store = {}

            def emit_a_group(tt, gi):
                emit_a_half(tt, gi, 0)
                emit_a_half(tt, gi, 1)

            # ---- attention phase ----
            held = None      # AV batch awaiting emission
            pending = None   # (ov, h, tt) awaiting normalization (DVE half)
            pendingB = None  # (o_sb, h, tt) awaiting transposes (PE half)

            def emit_avs(h_, upto=4):
                # whole-head AV, qc-major: PSUM allows only one open
                # accumulation group per bank, so each qc region's chunk
                # accumulation must be contiguous. Split qc 0-1 / 2-3 so the
                # part depending on the head's last causal mask goes out a
                # group later (mask latency hidden by the next head's QK).
                ov, h, tt, pts = h_[0], h_[1], h_[2], h_[3]
                lays = h_[5]
                for qc in range(h_[4], upto):
                    for jc in range(4 * tt + qc + 1):
                        g2, k = jc // 2, jc % 2
                        _, base_, a_ = lays[g2][k]
                        col = base_ + qc * 128 - a_
                        nc.tensor.matmul(
                            ov[:, qc * 65:(qc + 1) * 65],
                            pts[g2][:, col:col + 128],
                            v[jc][:, h * 65:(h + 1) * 65],
                            start=(jc == 0), stop=(jc == 4 * tt + qc),
                            skip_group_check=True,
                        )
                h_[4] = upto

            def emit_normA(pend):
                # DVE half of softmax normalization: 1/l then one broadcast
                # multiply into bf16 o_sb [q, 4*64]
                nonlocal held
                ov, h, tt = pend
                if held is not None and held[0] is ov:
                    emit_avs(held)
                    held = None
                ovv = ov[:].rearrange("p (q f) -> p q f", q=4, f=65)
                rl4 = pp.tile([128, 4], F32, tag="rl4", bufs=2)
                nc.vector.reciprocal(rl4[:], ovv[:, :, 64:65])
                o_sb = pp.tile([128, 256], BF16, tag="osb", bufs=2)
                nc.vector.tensor_tensor(
                    o_sb[:].rearrange("p (q f) -> p q f", q=4, f=64),
                    ovv[:, :, 0:64],
                    rl4[:].to_broadcast([128, 4, 64]),
                    mybir.AluOpType.mult,
                )
                return (o_sb, h, tt)

            def emit_normB(pendB):
                # PE half: transpose normalized O back to O^T layout in ot
                o_sb, h, tt = pendB
                p, e, i0 = h // 2, h % 2, tt * 512
                tps = psum.tile([64, 512], BF16, tag="yps", bufs=2,
                                name=f"tps{tt}{h}")
                for qc in range(4):
                    nc.tensor.matmul(
                        tps[:, qc * 128:(qc + 1) * 128],
                        o_sb[:, qc * 64:(qc + 1) * 64],
                        ident[:],
                        is_transpose=True, start=True, stop=True,
                        skip_group_check=True,
                    )
                nc.vector.tensor_copy(ot[p][e * 64:(e + 1) * 64, i0:i0 + 512],
                                      tps[:])

            def emit_outproj_group(tt, cp):
                i0 = tt * 512
                yps = psum.tile([128, 512], F32, tag="yps", bufs=2,
                                name=f"yps{tt}{cp}")
                for cl in range(4):
                    nc.tensor.matmul(
                        yps[:],
                        wot_t[:, cl * 1024 + cp * 128:cl * 1024 + (cp + 1) * 128],
                        ot[cl][:, i0:i0 + 512],
                        start=(cl == 0), stop=(cl == 3),
                        skip_group_check=True,
                    )
                ysb = pp.tile([128, 512], BF16, tag="ysb", bufs=4)
                nc.vector.tensor_scalar_add(ysb[:], yps[:], bo_sb[:, cp:cp + 1])
                nc.sync.dma_start(
                    out=y_part[tt, cp * 128:(cp + 1) * 128, :], in_=ysb[:]
                )

            def emit_outproj_partial(cp, half):
                i0 = 3 * 512
                cls = (0, 1) if half == 0 else (2, 3)
                yps = psum.tile([128, 512], F32, tag="yps", bufs=2,
                                name=f"op{half}_{cp}")
                for cl in cls:
                    nc.tensor.matmul(
                        yps[:],
                        wot_t[:, cl * 1024 + cp * 128:cl * 1024 + (cp + 1) * 128],
                        ot[cl][:, i0:i0 + 512],
                        start=(cl == cls[0]), stop=(cl == cls[1]),
                        skip_group_check=True,
                    )
                if half == 0:
                    nc.vector.tensor_scalar_add(
                        acc[:, cp * 512:(cp + 1) * 512], yps[:],
                        bo_sb[:, cp:cp + 1])
                else:
                    ysb = pp.tile([128, 512], BF16, tag="ysb", bufs=4)
                    nc.vector.tensor_tensor(
                        ysb[:], yps[:], acc[:, cp * 512:(cp + 1) * 512],
                        mybir.AluOpType.add)
                    nc.sync.dma_start(
                        out=y_part[3, cp * 128:(cp + 1) * 128, :], in_=ysb[:])

            def emit_rs(tt):
                if with_rs:
                    nc.gpsimd.collective_compute(
                        "ReduceScatter",
                        mybir.AluOpType.add,
                        replica_groups=RG,
                        ins=[y_part[tt]],
                        outs=[rs_out[tt]],
                    )
                    nc.sync.dma_start(out=y[tt], in_=rs_out[tt])
                else:
                    nc.sync.dma_start(out=y[tt], in_=y_part[tt, 0:512, :])

            # standalone projections for slab 0
            for gi in range(8):
                emit_a_group(0, gi)

            for tt in range(TTn):
                i0 = tt * 512
                n_jc = 4 * (tt + 1)
                ngroups = n_jc // 2
                if tt < TTn - 1:
                    issue_xts(tt + 1)
                for h in range(HC):
                    p, e = h // 2, h % 2
                    ov = psum.tile([128, 260], F32, tag="ov", bufs=1,
                                   name=f"ov{tt}{h}")
                    pts = []
                    lays = []
                    for g in range(ngroups):
                        if g == 2 * tt + 1:
                            # (k2,k3) diag pair packs to 384 cols: its own
                            # 1-bank tag keeps the main st rotation free
                            st = psum.tile([128, 384], F32, tag="st2", bufs=1,
                                           name=f"st{tt}{h}{g}")
                        else:
                            st = psum.tile([128, 1024], F32, tag="st", bufs=2,
                                           name=f"st{tt}{h}{g}")
                        # packed layout: each chunk's valid query range
                        # [a:512] stored contiguously, so exp touches no
                        # stale columns and diag groups shrink to 384 cols
                        lay = []
                        base = 0
                        for k in range(2):
                            jc = 2 * g + k
                            kb = jc - 4 * tt
                            a = kb * 128 if kb >= 0 else 0
                            lay.append((jc, base, a))
                            nc.tensor.matmul(
                                st[:, base:base + 512 - a],
                                kt[p][e * 64:(e + 1) * 64,
                                      jc * 128:(jc + 1) * 128],
                                qt[p][e * 64:(e + 1) * 64, i0 + a:i0 + 512],
                                start=True, stop=True,
                                skip_group_check=True,
                            )
                            base += 512 - a
                        # AV of the previously-held head keeps PE fed while
                        # ACT runs this group's exp; crosses head boundaries.
                        if held is not None:
                            if g == 0:
                                emit_avs(held, upto=2)
                            else:
                                emit_avs(held, upto=4)
                                held = None
                        if g == 0 and tt >= 1 and 1 <= h <= 4:
                            # previous slab's outproj + RS; RS two heads after
                            # the last y_part write so the collective's dep
                            # wait doesn't block Pool.SEQ (causal masks)
                            if h <= 2:
                                for g4 in range(2):
                                    emit_outproj_group(tt - 1, 4 * (h - 1) + g4)
                            elif h == 4:
                                emit_rs(tt - 1)
                        if g == 1:
                            if pending is not None:
                                pendingB = emit_normA(pending)
                                pending = None
                            # next slab's projections as PE filler (also hide
                            # normA's DVE latency before normB), split across
                            # g1/g2 so the filler granularity matches the
                            # per-group PE deficit
                            if tt < TTn - 1:
                                emit_a_half(tt + 1, h, 0)
                                if ngroups == 2:
                                    emit_a_half(tt + 1, h, 1)
                            elif h in (5, 6):
                                for cp in range(4 * (h - 5), 4 * (h - 4)):
                                    emit_outproj_partial(cp, 0)
                        if g == 2:
                            if pendingB is not None:
                                emit_normB(pendingB)
                                pendingB = None
                            if tt < TTn - 1:
                                emit_a_half(tt + 1, h, 1)
                            if tt >= 1 and 1 <= h <= 2:
                                for g4 in range(2, 4):
                                    emit_outproj_group(tt - 1, 4 * (h - 1) + g4)
                        pt = pp.tile([128, 1024], BF16, tag="pt", bufs=10,
                                     name=f"pt{tt}{h}{g}")
                        nc.scalar.activation(
                            pt[:, 0:base], st[:, 0:base],
                            mybir.ActivationFunctionType.Exp,
                        )
                        if lay[0][2] or lay[1][2] or 2 * g == 4 * tt:
                            # causal zeroing: in packed layout each diag
                            # chunk's triangle is its first 128 local cols,
                            # iota = i - p
                            for jc_, base_, a_ in lay:
                                if jc_ - 4 * tt < 0:
                                    continue
                                nc.gpsimd.affine_select(
                                    out=pt[:, base_:base_ + 128],
                                    in_=pt[:, base_:base_ + 128],
                                    compare_op=mybir.AluOpType.is_ge,
                                    fill=0.0, base=0,
                                    pattern=[[1, 128]],
                                    channel_multiplier=-1,
                                )
                        pts.append(pt)
                        lays.append(lay)
                    if pendingB is not None:  # ngroups == 2 has no g2 slot
                        emit_normB(pendingB)
                        pendingB = None
                    held = [ov, h, tt, pts, 0, lays]
                    pending = (ov, h, tt)
                if tt == TTn - 1:
                    if pending is not None:
                        emit_normB(emit_normA(pending))
                        pending = None
                    for cp in range(8):
                        emit_outproj_partial(cp, 1)
                    emit_rs(tt)

    nc.compile()
    return nc


_NC_CACHE = {}


def _get_nc(with_rs: bool = True):
    key = bool(with_rs)
    if key not in _NC_CACHE:
        _NC_CACHE[key] = build_nc(with_rs)
    return _NC_CACHE[key]


def make_in_maps(x, Wq, Wk, Wv, Wo, bo):
    import ml_dtypes
    bf16 = ml_dtypes.bfloat16

    x = np.asarray(x, dtype=np.float32)
    Wq = np.asarray(Wq, dtype=np.float32)
    Wk = np.asarray(Wk, dtype=np.float32)
    Wv = np.asarray(Wv, dtype=np.float32)
    Wo = np.asarray(Wo, dtype=np.float32)
    bo = np.asarray(bo, dtype=np.float32)

    scale = np.float32(C) ** np.float32(-0.5)
    in_maps = []
    for c in range(N_CORES):
        b, hoff = c // 2, (c % 2) * HC
        heads = slice(hoff, hoff + HC)
        xT_c = np.ascontiguousarray(x[b].T.astype(bf16))              # [C, T]
        wq_c = np.ascontiguousarray(
            np.concatenate(list(Wq[heads] * scale), axis=1).astype(bf16))
        wk_c = np.ascontiguousarray(
            np.concatenate(list(Wk[heads]), axis=1).astype(bf16))
        wv_c = np.ascontiguousarray(
            np.concatenate(list(Wv[heads]), axis=1).astype(bf16))
        wot_c = np.ascontiguousarray(
            Wo[:, hoff * D:(hoff + HC) * D].T.astype(bf16))           # [512, C]
        bo2_c = np.ascontiguousarray((bo / 2.0).reshape(8, 128).T)    # [128, 8]
        in_maps.append({
            "xT": xT_c, "wq": wq_c, "wk": wk_c, "wv": wv_c,
            "wot": wot_c, "bo2": bo2_c,
        })
    return in_maps


def kernel(x, Wq, Wk, Wv, Wo, bo):
    nc = _get_nc(with_rs=True)
    in_maps = make_in_maps(x, Wq, Wk, Wv, Wo, bo)
    # The axon-tunneled devices occasionally fail transiently
    # (NRT_EXEC_UNIT_UNRECOVERABLE / tunnel hangup); a retry recovers.
    last_err = None
    for _ in range(3):
        try:
            res = run_bass_kernel_spmd(nc, in_maps, list(range(N_CORES))).results
            break
        except Exception as e:  # noqa: BLE001
            last_err = e
            import time
            time.sleep(5)
    else:
        raise last_err

    out = np.empty((B, T, C), dtype=np.float32)
    for c in range(N_CORES):
        b, e = c // 2, c % 2
        yc = np.asarray(res[c]["y"], dtype=np.float32)  # [tt, c' slab, t]
        for tt in range(TTn):
            out[b, tt * 512:(tt + 1) * 512, e * 512:(e + 1) * 512] = yc[tt].T
    return out
